# revision 1
# baseline (speedup 1.0000x reference)
"""Trainium2 Bass kernel for a Mamba block (B=2, L=2048, d_model=1024,
d_inner=2048, d_state=16, d_conv=4, dt_rank=64), SPMD over 8 NeuronCores.

Sharding: 2 (batch) x 4 (d_inner shards of 512 channels). Each core computes
its batch's in_proj for its 512 channels (d-major layout: channels on SBUF
partitions, sequence on the free dim), the depthwise conv + silu, a partial
x_dbl that is AllReduce'd within each 4-core batch group, its local delta /
selective scan / gating, and a partial (L, d_model) output that the host sums.

The selective scan runs on the native tensor_tensor_scan instruction
(state = dA*state + dBu along the free dim). All 16 state dims are scanned
in ONE instruction per (d-tile, l-chunk) by laying the free axis out as
(n-outer, l-inner) segments and zeroing dA's first column per segment
(segment reset); the carried state is folded into dBu's first column.
"""
import os
import sys
from contextlib import ExitStack

import numpy as np

for _p in ("/opt/trn_rl_repo", "/root/.axon_site/_ro/trn_rl_repo"):
    if os.path.isdir(_p) and _p not in sys.path:
        sys.path.insert(0, _p)

import concourse.bass as bass
import concourse.mybir as mybir
import concourse.tile as tile
from concourse import bacc
from concourse.bass_utils import run_bass_kernel_spmd

F32 = mybir.dt.float32
CFG = {"cmul": "v", "dbu": "g", "bccopy": "a", "bc_pb": False, "psout_bufs": 3}
AF = mybir.ActivationFunctionType
OP = mybir.AluOpType

DM, DI, DS, DC, DR = 1024, 2048, 16, 4, 64
B, L = 2, 2048
NSH = 4            # d_inner shards per batch
DL = DI // NSH     # 512 channels per core
KT = DL // 128     # 4 partition tiles of channels
T = 256            # scan l-chunk
NCH = L // T       # 8 chunks
PC = 512           # phase-1/2/3 l-chunk
NPC = L // PC      # 4


def build_program(n_reps: int = 1, use_collective: bool = True, skip=frozenset()):
    nc = bacc.Bacc("TRN2", target_bir_lowering=False)
    hsT = nc.declare_dram_parameter("hsT", [DM, L], F32, isOutput=False)
    wix = nc.declare_dram_parameter("wix", [DM, DL], F32, isOutput=False)
    wiz = nc.declare_dram_parameter("wiz", [DM, DL], F32, isOutput=False)
    wc = nc.declare_dram_parameter("wc", [DL, DC], F32, isOutput=False)
    bcv = nc.declare_dram_parameter("bcv", [DL, 1], F32, isOutput=False)
    nbcv = nc.declare_dram_parameter("nbcv", [DL, 1], F32, isOutput=False)
    wx = nc.declare_dram_parameter("wx", [DL, 96], F32, isOutput=False)
    wdt = nc.declare_dram_parameter("wdt", [DR, DL], F32, isOutput=False)
    bdt = nc.declare_dram_parameter("bdt", [DL, 1], F32, isOutput=False)
    asc = nc.declare_dram_parameter("asc", [DL, DS], F32, isOutput=False)
    dpar = nc.declare_dram_parameter("dpar", [DL, 1], F32, isOutput=False)
    wout = nc.declare_dram_parameter("wout", [DL, DM], F32, isOutput=False)
    sel = nc.declare_dram_parameter("sel", [2 * DS, 2 * DS * 128], F32,
                                    isOutput=False)
    outp = nc.declare_dram_parameter("outp", [L, DM], F32, isOutput=True)

    with tile.TileContext(nc) as tc, ExitStack() as ctx:
        def emit_once():
            dram = ctx.enter_context(tc.tile_pool(name="dram", bufs=1, space="DRAM"))
            z_dram = dram.tile([DL, L], F32)
            delta_dram = dram.tile([DL, L], F32)
            xd_bounce = [dram.tile([96, PC], F32, name=f"xdb{c}") for c in range(NPC)]
            xd_red = [dram.tile([96, PC], F32, name=f"xdr{c}") for c in range(NPC)]

            consts = ctx.enter_context(tc.tile_pool(name="consts", bufs=1))
            # per-k tiles packed side by side: wc_t[:, k*DC:(k+1)*DC]
            wc_t = consts.tile([128, DC * KT], F32, tag="wc")
            bcv_t = consts.tile([128, KT], F32, tag="bcv")
            nbcv_t = consts.tile([128, KT], F32, tag="nbcv")
            bdt_t = consts.tile([128, KT], F32, tag="bdt")
            asc_t = consts.tile([128, DS * KT], F32, tag="asc")
            dpar_t = consts.tile([128, KT], F32, tag="dpar")
            if not CFG.get("bc_pb", True):
                sel_t = consts.tile([2 * DS, 2 * DS * 128], F32, tag="sel")
                nc.sync.dma_start(sel_t[:], sel[:])
            for k in range(KT):
                ksl = slice(128 * k, 128 * (k + 1))
                nc.sync.dma_start(wc_t[:, DC * k:DC * (k + 1)], wc[ksl, :])
                nc.sync.dma_start(bcv_t[:, k:k + 1], bcv[ksl, :])
                nc.sync.dma_start(nbcv_t[:, k:k + 1], nbcv[ksl, :])
                nc.sync.dma_start(bdt_t[:, k:k + 1], bdt[ksl, :])
                nc.sync.dma_start(asc_t[:, DS * k:DS * (k + 1)], asc[ksl, :])
                nc.sync.dma_start(dpar_t[:, k:k + 1], dpar[ksl, :])

            persist = ctx.enter_context(tc.tile_pool(name="persist", bufs=1))
            xs_t = [persist.tile([128, L], F32, tag=f"xs{k}", name=f"xs{k}") for k in range(KT)]
            state_t = persist.tile([128, DS * KT], F32, tag="state")
            nc.vector.memset(state_t[:], 0.0)
            wout_t = [persist.tile([128, DM], F32, tag=f"wout{k}", name=f"wout{k}") for k in range(KT)]
            for k in range(KT):
                nc.sync.dma_start(wout_t[k][:], wout[128 * k:128 * (k + 1), :])

            # ---------------- Phase 1: in_proj (x, z) ----------------
            with ExitStack() as p1:
                wpool = p1.enter_context(tc.tile_pool(name="w_in", bufs=1))
                wix_t = [wpool.tile([128, DL], F32, tag=f"wix{kk}", name=f"wix{kk}") for kk in range(8)]
                wiz_t = [wpool.tile([128, DL], F32, tag=f"wiz{kk}", name=f"wiz{kk}") for kk in range(8)]
                for kk in range(8):
                    nc.sync.dma_start(wix_t[kk][:], wix[128 * kk:128 * (kk + 1), :])
                    nc.sync.dma_start(wiz_t[kk][:], wiz[128 * kk:128 * (kk + 1), :])
                xpad_p = p1.enter_context(tc.tile_pool(name="xpad", bufs=1))
                xpad = [xpad_p.tile([128, L + 3], F32, tag=f"xp{k}", name=f"xp{k}") for k in range(KT)]
                for k in range(KT):
                    nc.vector.memset(xpad[k][:, 0:1], 0.0)
                    nc.vector.memset(xpad[k][:, L + 1:L + 3], 0.0)
                hs_pool = p1.enter_context(tc.tile_pool(name="hs", bufs=3))
                ps1 = p1.enter_context(
                    tc.tile_pool(name="ps1", bufs=1, space="PSUM"))
                zs_pool = p1.enter_context(tc.tile_pool(name="zsil", bufs=3))

                for c in range(NPC):
                    lsl = slice(PC * c, PC * (c + 1))
                    px = [ps1.tile([128, PC], F32, tag=f"px{k}", name=f"px{k}") for k in range(KT)]
                    pz = [ps1.tile([128, PC], F32, tag=f"pz{k}", name=f"pz{k}") for k in range(KT)]
                    for kk in range(8):
                        h = hs_pool.tile([128, PC], F32, tag="hs")
                        nc.sync.dma_start(h[:], hsT[128 * kk:128 * (kk + 1), lsl])
                        for k in range(KT):
                            ksl = slice(128 * k, 128 * (k + 1))
                            nc.tensor.matmul(px[k][:], wix_t[kk][:, ksl], h[:],
                                             start=(kk == 0), stop=(kk == 7))
                            nc.tensor.matmul(pz[k][:], wiz_t[kk][:, ksl], h[:],
                                             start=(kk == 0), stop=(kk == 7))
                    for k in range(KT):
                        nc.scalar.copy(xpad[k][:, 1 + PC * c:1 + PC * (c + 1)],
                                       px[k][:])
                        if 'zsilu' in skip: continue
                        zc = zs_pool.tile([128, PC], F32, tag="zc")
                        nc.scalar.copy(zc[:], pz[k][:])
                        zs = zs_pool.tile([128, PC], F32, tag="zs")
                        nc.scalar.activation(zs[:], zc[:], AF.Exp, scale=-1.0)
                        nc.scalar.activation(zs[:], zs[:], AF.Ln, bias=1.0)
                        nc.scalar.activation(zs[:], zs[:], AF.Exp, scale=-1.0)
                        zt = zs_pool.tile([128, PC], F32, tag="z")
                        nc.vector.tensor_tensor(zt[:], zc[:], zs[:], OP.mult)
                        nc.sync.dma_start(z_dram[128 * k:128 * (k + 1), lsl], zt[:])

                # ---------------- Phase 1b: conv + silu -> xs ----------------
                cvp = p1.enter_context(tc.tile_pool(name="cv", bufs=2))
                for k in range(0 if 'conv' in skip else KT):
                    t0 = cvp.tile([128, L], F32, tag="cv")
                    nc.vector.tensor_scalar(t0[:], xpad[k][:, 0:L],
                                            wc_t[:, DC * k:DC * k + 1], None, OP.mult)
                    t1 = cvp.tile([128, L], F32, tag="cv")
                    nc.vector.scalar_tensor_tensor(
                        t1[:], xpad[k][:, 1:1 + L], wc_t[:, DC * k + 1:DC * k + 2],
                        t0[:], OP.mult, OP.add)
                    t2 = cvp.tile([128, L], F32, tag="cv")
                    nc.vector.scalar_tensor_tensor(
                        t2[:], xpad[k][:, 2:2 + L], wc_t[:, DC * k + 2:DC * k + 3],
                        t1[:], OP.mult, OP.add)
                    t3 = cvp.tile([128, L], F32, tag="cv")
                    nc.vector.scalar_tensor_tensor(
                        t3[:], xpad[k][:, 3:3 + L], wc_t[:, DC * k + 3:DC * k + 4],
                        t2[:], OP.mult, OP.add)
                    sg = cvp.tile([128, L], F32, tag="sg")
                    nc.scalar.activation(sg[:], t3[:], AF.Exp, scale=-1.0,
                                         bias=nbcv_t[:, k:k + 1])
                    nc.scalar.activation(sg[:], sg[:], AF.Ln, bias=1.0)
                    nc.scalar.activation(sg[:], sg[:], AF.Exp, scale=-1.0)
                    nc.vector.scalar_tensor_tensor(
                        xs_t[k][:], t3[:], bcv_t[:, k:k + 1], sg[:],
                        OP.add, OP.mult)

            # ---------------- Phase 2: x_dbl partial + AllReduce ----------------
            with ExitStack() as p2:
                wx_p = p2.enter_context(tc.tile_pool(name="wx", bufs=1))
                wx_t = [wx_p.tile([128, 96], F32, tag=f"wx{k}", name=f"wx{k}") for k in range(KT)]
                for k in range(KT):
                    nc.sync.dma_start(wx_t[k][:], wx[128 * k:128 * (k + 1), :])
                ps2 = p2.enter_context(tc.tile_pool(name="ps2", bufs=2, space="PSUM"))
                xdp = p2.enter_context(tc.tile_pool(name="xdp", bufs=2))
                for c in range(NPC):
                    lsl = slice(PC * c, PC * (c + 1))
                    pxd = ps2.tile([96, PC], F32, tag="pxd")
                    for k in range(KT):
                        nc.tensor.matmul(pxd[:], wx_t[k][:], xs_t[k][:, lsl],
                                         start=(k == 0), stop=(k == KT - 1))
                    xt = xdp.tile([96, PC], F32, tag="xdp")
                    nc.scalar.copy(xt[:], pxd[:])
                    nc.sync.dma_start(xd_bounce[c][:], xt[:])
                    if use_collective:
                        nc.gpsimd.collective_compute(
                            "AllReduce", OP.add,
                            replica_groups=[[0, 1, 2, 3], [4, 5, 6, 7]],
                            ins=[xd_bounce[c].opt()], outs=[xd_red[c].opt()])
                    else:
                        nc.sync.dma_start(xd_red[c][:], xd_bounce[c][:])
            # B/C rows staged at partitions 0..31 for the selection broadcasts
            bcT_p = ctx.enter_context(tc.tile_pool(name="bcT", bufs=1))
            bcT = bcT_p.tile([2 * DS, L], F32, tag="bcT")
            for c in range(NPC):
                nc.sync.dma_start(bcT[:, PC * c:PC * (c + 1)],
                                  xd_red[c][DR:DR + 2 * DS, :])

            # ---------------- Phase 3: delta = softplus(dt) ----------------
            with ExitStack() as p3:
                wdt_p = p3.enter_context(tc.tile_pool(name="wdt", bufs=1))
                wdt_t = wdt_p.tile([128, DL], F32, tag="wdt")
                nc.sync.dma_start(wdt_t[0:DR, :], wdt[:])
                ps3 = p3.enter_context(tc.tile_pool(name="ps3", bufs=2, space="PSUM"))
                dchunk = p3.enter_context(tc.tile_pool(name="dch", bufs=2))
                xdb_p = p3.enter_context(tc.tile_pool(name="xdb", bufs=2))
                for c in range(NPC):
                    lsl = slice(PC * c, PC * (c + 1))
                    xdb = xdb_p.tile([DR, PC], F32, tag="xdb")
                    nc.sync.dma_start(xdb[:], xd_red[c][0:DR, :])
                    for k in range(KT):
                        pdt = ps3.tile([128, PC], F32, tag="pdt")
                        nc.tensor.matmul(pdt[:], wdt_t[0:DR, 128 * k:128 * (k + 1)],
                                         xdb[:], start=True, stop=True)
                        dt = dchunk.tile([128, PC], F32, tag="dt")
                        nc.scalar.activation(dt[:], pdt[:], AF.Exp,
                                             bias=bdt_t[:, k:k + 1])
                        nc.scalar.activation(dt[:], dt[:], AF.Ln, bias=1.0)
                        nc.sync.dma_start(delta_dram[128 * k:128 * (k + 1), lsl], dt[:])

            # ---------------- Phase M: scan middle + output ----------------
            mid = ctx.enter_context(tc.tile_pool(name="mid", bufs=CFG.get("mid_bufs", 2)))
            s_pool = ctx.enter_context(tc.tile_pool(name="spool", bufs=CFG.get("s_bufs", 1)))
            bc_p = ctx.enter_context(tc.tile_pool(name="bc", bufs=1))
            ps_bc = ctx.enter_context(tc.tile_pool(name="psbc", bufs=2, space="PSUM"))
            ps_out = ctx.enter_context(tc.tile_pool(name="psout", bufs=CFG.get("psout_bufs", 2), space="PSUM"))
            dre = ctx.enter_context(tc.tile_pool(name="dre", bufs=CFG.get("dre_bufs", 2)))
            ych = ctx.enter_context(tc.tile_pool(name="ych", bufs=CFG.get("ych_bufs", 2)))
            outc = ctx.enter_context(tc.tile_pool(name="outc", bufs=1))

            for c in range(NCH):
                lsl = slice(T * c, T * (c + 1))
                # B_bc / C_bc patterns [128, 16*T], (n-outer, l-inner)
                b_bc = bc_p.tile([128, DS * T], F32, tag="b_bc")
                c_bc = bc_p.tile([128, DS * T], F32, tag="c_bc")
                for half, dst in (() if 'bc' in skip else ((0, b_bc), (1, c_bc))):
                    if CFG.get("bc_pb", True):
                        for n in range(DS):
                            nc.gpsimd.partition_broadcast(
                                dst[:, T * n:T * (n + 1)],
                                bcT[16 * half + n:16 * half + n + 1, lsl])
                        continue
                    for g in range(DS // 2):  # two n per psum tile
                        pb = ps_bc.tile([128, 2 * T], F32, tag="pb")
                        for j in range(2):
                            n = 16 * half + 2 * g + j
                            nc.tensor.matmul(pb[:, T * j:T * (j + 1)],
                                             sel_t[:, 128 * n:128 * (n + 1)],
                                             bcT[:, lsl],
                                             start=True, stop=True)
                        (nc.scalar.copy if CFG["bccopy"] == "a" else
                         nc.gpsimd.tensor_copy)(dst[:, 2 * T * g:2 * T * (g + 1)], pb[:])

                for k in range(KT):
                    ksl = slice(128 * k, 128 * (k + 1))
                    nsl = slice(DS * k, DS * (k + 1))
                    d_ch = dre.tile([128, T], F32, tag="dch")
                    nc.sync.dma_start(d_ch[:], delta_dram[ksl, lsl])
                    # dA[n*T+l] = exp(A[d,n] * delta[d,l])
                    dA = mid.tile([128, DS * T], F32, tag="dA")
                    if 'dA' not in skip:
                      for n in range(DS):
                        nc.scalar.activation(dA[:, T * n:T * (n + 1)], d_ch[:],
                                             AF.Exp,
                                             scale=asc_t[:, DS * k + n:DS * k + n + 1])
                    # dBu = (delta*x) expanded  *  B_bc
                    dx = dre.tile([128, T], F32, tag="dx")
                    nc.vector.tensor_tensor(dx[:], d_ch[:], xs_t[k][:, lsl], OP.mult)
                    dBu = mid.tile([128, DS * T], F32, tag="dBu")
                    if 'dbu' not in skip:
                        engd = nc.vector if CFG["dbu"] == "v" else nc.gpsimd
                        engd.tensor_tensor(
                            dBu[:].rearrange("p (n l) -> p n l", n=DS),
                            dx[:, None, :].to_broadcast([128, DS, T]),
                            b_bc[:].rearrange("p (n l) -> p n l", n=DS),
                            OP.mult)
                    # fold carried state into first column of each segment
                    fx = dre.tile([128, DS], F32, tag="fx")
                    nc.vector.tensor_tensor(fx[:], dA[:, 0:DS * T:T],
                                            state_t[:, nsl], OP.mult)
                    nc.vector.tensor_tensor(dBu[:, 0:DS * T:T], dBu[:, 0:DS * T:T],
                                            fx[:], OP.add)
                    nc.vector.memset(dA[:, 0:DS * T:T], 0.0)
                    # the scan: state = dA*state + dBu over the whole (n,l) axis
                    s_t = s_pool.tile([128, DS * T], F32, tag="s")
                    if 'scan' not in skip:
                        se = (nc.gpsimd if (CFG.get("scan_split") and
                                            k >= CFG["scan_split"]) else nc.vector)
                        se.tensor_tensor_scan(s_t[:], dA[:], dBu[:], 0.0,
                                              OP.mult, OP.add)
                        se.tensor_copy(state_t[:, nsl], s_t[:, T - 1:DS * T:T])
                    # y = sum_n s*C
                    if 'cmul' not in skip:
                        eng = nc.vector if CFG["cmul"] == "v" else nc.gpsimd
                        eng.tensor_tensor(s_t[:], s_t[:], c_bc[:], OP.mult)
                    y_r = ych.tile([128, T], F32, tag="yr")
                    if 'reduce' not in skip:
                        nc.vector.tensor_reduce(
                            y_r[:], s_t[:].rearrange("p (n l) -> p l n", n=DS),
                            axis=mybir.AxisListType.X, op=OP.add)
                    # skip + gate
                    nc.vector.scalar_tensor_tensor(
                        y_r[:], xs_t[k][:, lsl], dpar_t[:, k:k + 1], y_r[:],
                        OP.mult, OP.add)
                    z_ch = dre.tile([128, T], F32, tag="zch")
                    nc.sync.dma_start(z_ch[:], z_dram[ksl, lsl])
                    g_t = ych.tile([128, T], F32, tag=f"g{k}")
                    nc.vector.tensor_tensor(g_t[:], y_r[:], z_ch[:], OP.mult)
                    if k == 0:
                        g_list = []
                    g_list.append(g_t)

                # out_proj for this chunk: out[l, :] += g^T @ W_out
                for h in range(0 if 'out' in skip else T // 128):
                    po0 = ps_out.tile([128, 512], F32, tag="po0")
                    po1 = ps_out.tile([128, 512], F32, tag="po1")
                    msl = slice(128 * h, 128 * (h + 1))
                    for k in range(KT):
                        nc.tensor.matmul(po0[:], g_list[k][:, msl],
                                         wout_t[k][:, 0:512],
                                         start=(k == 0), stop=(k == KT - 1))
                    for k in range(KT):
                        nc.tensor.matmul(po1[:], g_list[k][:, msl],
                                         wout_t[k][:, 512:1024],
                                         start=(k == 0), stop=(k == KT - 1))
                    ot = outc.tile([128, DM], F32, tag="ot")
                    nc.scalar.copy(ot[:, 0:512], po0[:])
                    nc.scalar.copy(ot[:, 512:1024], po1[:])
                    nc.sync.dma_start(outp[T * c + 128 * h:T * c + 128 * (h + 1), :],
                                      ot[:])
        for _rep in range(n_reps):
            emit_once()
    nc.compile()
    return nc


_NC_CACHE = None


def kernel(**inputs) -> np.ndarray:
    global _NC_CACHE
    hs = np.ascontiguousarray(inputs["hidden_states"], np.float32)
    W_in = np.asarray(inputs["W_in"], np.float32)
    W_conv = np.asarray(inputs["W_conv"], np.float32)
    b_conv = np.asarray(inputs["b_conv"], np.float32)
    W_x = np.asarray(inputs["W_x"], np.float32)
    W_dt = np.asarray(inputs["W_dt"], np.float32)
    b_dt = np.asarray(inputs["b_dt"], np.float32)
    A_log = np.asarray(inputs["A_log"], np.float32)
    D_param = np.asarray(inputs["D_param"], np.float32)
    W_out = np.asarray(inputs["W_out"], np.float32)
    A = -np.exp(A_log.astype(np.float64)).astype(np.float32)    # (DI, DS)
    sel_mat = np.zeros((2 * DS, 2 * DS * 128), np.float32)
    for n in range(2 * DS):
        sel_mat[n, 128 * n:128 * (n + 1)] = 1.0

    in_maps = []
    for cid in range(8):
        b, s = cid // NSH, cid % NSH
        sh = slice(DL * s, DL * (s + 1))
        in_maps.append({
            "hsT": np.ascontiguousarray(hs[b].T),
            "wix": np.ascontiguousarray(W_in[:, 2 * DL * s:2 * DL * (s + 1):2]),
            "wiz": np.ascontiguousarray(W_in[:, 2 * DL * s + 1:2 * DL * (s + 1) + 1:2]),
            "wc": np.ascontiguousarray(W_conv[:, 0, sh].T),
            "bcv": np.ascontiguousarray(b_conv[sh].reshape(DL, 1)),
            "nbcv": np.ascontiguousarray(-b_conv[sh].reshape(DL, 1)),
            "wx": np.ascontiguousarray(W_x[sh, :]),
            "wdt": np.ascontiguousarray(W_dt[:, sh]),
            "bdt": np.ascontiguousarray(b_dt[sh].reshape(DL, 1)),
            "asc": np.ascontiguousarray(A[sh, :]),
            "dpar": np.ascontiguousarray(D_param[sh].reshape(DL, 1)),
            "wout": np.ascontiguousarray(W_out[sh, :]),
            "sel": sel_mat,
        })

    global _LAST_IN_MAPS
    _LAST_IN_MAPS = in_maps
    if _NC_CACHE is None:
        _NC_CACHE = build_program()
    res = run_bass_kernel_spmd(_NC_CACHE, in_maps, list(range(8)))
    out = np.zeros((B, L, DM), np.float32)
    for cid in range(8):
        out[cid // NSH] += res.results[cid]["outp"]
    return out


if __name__ == "__main__":
    rng = np.random.default_rng(0)
    dummy = {
        "hidden_states": rng.standard_normal((B, L, DM), dtype=np.float32),
        "W_in": rng.standard_normal((DM, 2 * DI), dtype=np.float32) * 0.03,
        "W_conv": rng.standard_normal((DC, 1, DI), dtype=np.float32) * 0.5,
        "b_conv": np.zeros((DI,), np.float32),
        "W_x": rng.standard_normal((DI, DR + 2 * DS), dtype=np.float32) * 0.02,
        "W_dt": rng.standard_normal((DR, DI), dtype=np.float32) * 0.12,
        "b_dt": rng.standard_normal((DI,), dtype=np.float32) * 0.01,
        "A_log": np.log(np.broadcast_to(np.arange(1, DS + 1, dtype=np.float32),
                                        (DI, DS))).copy(),
        "D_param": np.ones((DI,), np.float32),
        "W_out": rng.standard_normal((DI, DM), dtype=np.float32) * 0.03,
    }
    out = kernel(**dummy)
    print("out", out.shape, out.dtype, np.abs(out).max())



# revision 9
# speedup vs baseline: 1.0061x; 1.0061x over previous
"""Trainium2 Bass kernel for a Mamba block (B=2, L=2048, d_model=1024,
d_inner=2048, d_state=16, d_conv=4, dt_rank=64), SPMD over 8 NeuronCores.

Sharding: 2 (batch) x 4 (d_inner shards of 512 channels). Each core computes
its batch's in_proj for its 512 channels (d-major layout: channels on SBUF
partitions, sequence on the free dim), the depthwise conv + silu, a partial
x_dbl that is AllReduce'd within each 4-core batch group, its local delta /
selective scan / gating, and a partial (L, d_model) output that the host sums.

The selective scan runs on the native tensor_tensor_scan instruction
(state = dA*state + dBu along the free dim). All 16 state dims are scanned
in ONE instruction per (d-tile, l-chunk) by laying the free axis out as
(n-outer, l-inner) segments and zeroing dA's first column per segment
(segment reset); the carried state is folded into dBu's first column.
"""
import os
import sys
from contextlib import ExitStack

import numpy as np

for _p in ("/opt/trn_rl_repo", "/root/.axon_site/_ro/trn_rl_repo"):
    if os.path.isdir(_p) and _p not in sys.path:
        sys.path.insert(0, _p)

import concourse.bass as bass
import concourse.mybir as mybir
import concourse.tile as tile
from concourse import bacc
from concourse.bass_utils import run_bass_kernel_spmd

F32 = mybir.dt.float32
F32R = mybir.dt.float32r
CFG = {"cmul": "v", "dbu": "g", "bccopy": "a", "bc_pb": False, "psout_bufs": 3}
AF = mybir.ActivationFunctionType
OP = mybir.AluOpType


class PinnedBacc(bacc.Bacc):
    """Bacc whose act-table-load pass only considers table sets that serve
    every activation function this kernel uses, so the fixpoint settles on a
    single LoadActFuncSet instead of toggling exp<->ln sets per phase."""

    ACT_KEEP = ("natural_log_exp_and_others",)

    def insert_act_table_loads(self):
        import bass_rust as _bass_rust
        from concourse.hw_specs import get_activation_tables

        tables = list(get_activation_tables(self.m.arch).items())
        pinned = [(nm, fs if nm in self.ACT_KEEP else set()) for nm, fs in tables]
        _bass_rust.insert_act_table_loads(self, pinned)

DM, DI, DS, DC, DR = 1024, 2048, 16, 4, 64
B, L = 2, 2048
NSH = 4            # d_inner shards per batch
DL = DI // NSH     # 512 channels per core
KT = DL // 128     # 4 partition tiles of channels
T = 256            # scan l-chunk
NCH = L // T       # 8 chunks
PC = 512           # phase-1/2/3 l-chunk
NPC = L // PC      # 4


def build_program(n_reps: int = 1, use_collective: bool = True, skip=frozenset()):
    nc = PinnedBacc("TRN2", target_bir_lowering=False)
    hsT = nc.declare_dram_parameter("hsT", [DM, L], F32, isOutput=False)
    wix = nc.declare_dram_parameter("wix", [DM, DL], F32, isOutput=False)
    wiz = nc.declare_dram_parameter("wiz", [DM, DL], F32, isOutput=False)
    wc = nc.declare_dram_parameter("wc", [DL, DC], F32, isOutput=False)
    bcv = nc.declare_dram_parameter("bcv", [DL, 1], F32, isOutput=False)
    nbcv = nc.declare_dram_parameter("nbcv", [DL, 1], F32, isOutput=False)
    wx = nc.declare_dram_parameter("wx", [DL, 96], F32, isOutput=False)
    wdt = nc.declare_dram_parameter("wdt", [DR, DL], F32, isOutput=False)
    bdt = nc.declare_dram_parameter("bdt", [DL, 1], F32, isOutput=False)
    asc = nc.declare_dram_parameter("asc", [DL, DS], F32, isOutput=False)
    dpar = nc.declare_dram_parameter("dpar", [DL, 1], F32, isOutput=False)
    wout = nc.declare_dram_parameter("wout", [DL, DM], F32, isOutput=False)
    sel = nc.declare_dram_parameter("sel", [2 * DS, 2 * DS * 128], F32,
                                    isOutput=False)
    outp = nc.declare_dram_parameter("outp", [L, DM], F32, isOutput=True)

    with tile.TileContext(nc) as tc, ExitStack() as ctx:
        def emit_once():
            dram = ctx.enter_context(tc.tile_pool(name="dram", bufs=1, space="DRAM"))
            z_dram = dram.tile([DL, L], F32)
            delta_dram = dram.tile([DL, L], F32)
            xd_bounce = [dram.tile([96, PC], F32, name=f"xdb{c}") for c in range(NPC)]
            xd_red = [dram.tile([96, PC], F32, name=f"xdr{c}") for c in range(NPC)]

            consts = ctx.enter_context(tc.tile_pool(name="consts", bufs=1))
            # per-k tiles packed side by side: wc_t[:, k*DC:(k+1)*DC]
            wc_t = consts.tile([128, DC * KT], F32, tag="wc")
            bcv_t = consts.tile([128, KT], F32, tag="bcv")
            nbcv_t = consts.tile([128, KT], F32, tag="nbcv")
            bdt_t = consts.tile([128, KT], F32, tag="bdt")
            asc_t = consts.tile([128, DS * KT], F32, tag="asc")
            dpar_t = consts.tile([128, KT], F32, tag="dpar")
            if not CFG.get("bc_pb", True):
                sel_t = consts.tile([2 * DS, 2 * DS * 128], F32, tag="sel")
                nc.sync.dma_start(sel_t[:].bitcast(F32R), sel[:].bitcast(F32R))
            for k in range(KT):
                ksl = slice(128 * k, 128 * (k + 1))
                nc.sync.dma_start(wc_t[:, DC * k:DC * (k + 1)], wc[ksl, :])
                nc.sync.dma_start(bcv_t[:, k:k + 1], bcv[ksl, :])
                nc.sync.dma_start(nbcv_t[:, k:k + 1], nbcv[ksl, :])
                nc.sync.dma_start(bdt_t[:, k:k + 1], bdt[ksl, :])
                nc.sync.dma_start(asc_t[:, DS * k:DS * (k + 1)], asc[ksl, :])
                nc.sync.dma_start(dpar_t[:, k:k + 1], dpar[ksl, :])

            persist = ctx.enter_context(tc.tile_pool(name="persist", bufs=1))
            xs_t = [persist.tile([128, L], F32, tag=f"xs{k}", name=f"xs{k}") for k in range(KT)]
            state_t = persist.tile([128, DS * KT], F32, tag="state")
            nc.vector.memset(state_t[:], 0.0)
            wout_t = [persist.tile([128, DM], F32, tag=f"wout{k}", name=f"wout{k}") for k in range(KT)]
            for k in range(KT):
                nc.sync.dma_start(wout_t[k][:].bitcast(F32R), wout[128 * k:128 * (k + 1), :].bitcast(F32R))

            # ---------------- Phase 1: in_proj (x, z) ----------------
            with ExitStack() as p1:
                wpool = p1.enter_context(tc.tile_pool(name="w_in", bufs=1))
                wix_t = [wpool.tile([128, DL], F32, tag=f"wix{kk}", name=f"wix{kk}") for kk in range(8)]
                wiz_t = [wpool.tile([128, DL], F32, tag=f"wiz{kk}", name=f"wiz{kk}") for kk in range(8)]
                for kk in range(8):
                    nc.sync.dma_start(wix_t[kk][:].bitcast(F32R), wix[128 * kk:128 * (kk + 1), :].bitcast(F32R))
                    nc.sync.dma_start(wiz_t[kk][:].bitcast(F32R), wiz[128 * kk:128 * (kk + 1), :].bitcast(F32R))
                xpad_p = p1.enter_context(tc.tile_pool(name="xpad", bufs=1))
                xpad = [xpad_p.tile([128, L + 3], F32, tag=f"xp{k}", name=f"xp{k}") for k in range(KT)]
                for k in range(KT):
                    nc.vector.memset(xpad[k][:, 0:1], 0.0)
                    nc.vector.memset(xpad[k][:, L + 1:L + 3], 0.0)
                hs_pool = p1.enter_context(tc.tile_pool(name="hs", bufs=3))
                ps1 = p1.enter_context(
                    tc.tile_pool(name="ps1", bufs=1, space="PSUM"))
                zs_pool = p1.enter_context(tc.tile_pool(name="zsil", bufs=3))

                for c in range(NPC):
                    lsl = slice(PC * c, PC * (c + 1))
                    px = [ps1.tile([128, PC], F32, tag=f"px{k}", name=f"px{k}") for k in range(KT)]
                    pz = [ps1.tile([128, PC], F32, tag=f"pz{k}", name=f"pz{k}") for k in range(KT)]
                    for kk in range(8):
                        h = hs_pool.tile([128, PC], F32, tag="hs")
                        nc.sync.dma_start(h[:].bitcast(F32R), hsT[128 * kk:128 * (kk + 1), lsl].bitcast(F32R))
                        for k in range(KT):
                            ksl = slice(128 * k, 128 * (k + 1))
                            nc.tensor.matmul(px[k][:], wix_t[kk][:, ksl].bitcast(F32R),
                                             h[:].bitcast(F32R),
                                             start=(kk == 0), stop=(kk == 7))
                            nc.tensor.matmul(pz[k][:], wiz_t[kk][:, ksl].bitcast(F32R),
                                             h[:].bitcast(F32R),
                                             start=(kk == 0), stop=(kk == 7))
                    for k in range(KT):
                        nc.scalar.copy(xpad[k][:, 1 + PC * c:1 + PC * (c + 1)],
                                       px[k][:])
                        if 'zsilu' in skip: continue
                        zc = zs_pool.tile([128, PC], F32, tag="zc")
                        nc.scalar.copy(zc[:], pz[k][:])
                        zs = zs_pool.tile([128, PC], F32, tag="zs")
                        nc.scalar.activation(zs[:], zc[:], AF.Exp, scale=-1.0)
                        nc.scalar.activation(zs[:], zs[:], AF.Ln, bias=1.0)
                        nc.scalar.activation(zs[:], zs[:], AF.Exp, scale=-1.0)
                        zt = zs_pool.tile([128, PC], F32, tag="z")
                        nc.vector.tensor_tensor(zt[:], zc[:], zs[:], OP.mult)
                        nc.sync.dma_start(z_dram[128 * k:128 * (k + 1), lsl], zt[:])

                # ---------------- Phase 1b: conv + silu -> xs ----------------
                cvp = p1.enter_context(tc.tile_pool(name="cv", bufs=2))
                for k in range(0 if 'conv' in skip else KT):
                    t0 = cvp.tile([128, L], F32, tag="cv")
                    nc.vector.tensor_scalar(t0[:], xpad[k][:, 0:L],
                                            wc_t[:, DC * k:DC * k + 1], None, OP.mult)
                    t1 = cvp.tile([128, L], F32, tag="cv")
                    nc.vector.scalar_tensor_tensor(
                        t1[:], xpad[k][:, 1:1 + L], wc_t[:, DC * k + 1:DC * k + 2],
                        t0[:], OP.mult, OP.add)
                    t2 = cvp.tile([128, L], F32, tag="cv")
                    nc.vector.scalar_tensor_tensor(
                        t2[:], xpad[k][:, 2:2 + L], wc_t[:, DC * k + 2:DC * k + 3],
                        t1[:], OP.mult, OP.add)
                    t3 = cvp.tile([128, L], F32, tag="cv")
                    nc.vector.scalar_tensor_tensor(
                        t3[:], xpad[k][:, 3:3 + L], wc_t[:, DC * k + 3:DC * k + 4],
                        t2[:], OP.mult, OP.add)
                    sg = cvp.tile([128, L], F32, tag="sg")
                    nc.scalar.activation(sg[:], t3[:], AF.Exp, scale=-1.0,
                                         bias=nbcv_t[:, k:k + 1])
                    nc.scalar.activation(sg[:], sg[:], AF.Ln, bias=1.0)
                    nc.scalar.activation(sg[:], sg[:], AF.Exp, scale=-1.0)
                    nc.vector.scalar_tensor_tensor(
                        xs_t[k][:].bitcast(F32R), t3[:], bcv_t[:, k:k + 1], sg[:],
                        OP.add, OP.mult)

            # ---------------- Phase 2: x_dbl partial + AllReduce ----------------
            with ExitStack() as p2:
                wx_p = p2.enter_context(tc.tile_pool(name="wx", bufs=1))
                wx_t = [wx_p.tile([128, 96], F32, tag=f"wx{k}", name=f"wx{k}") for k in range(KT)]
                for k in range(KT):
                    nc.sync.dma_start(wx_t[k][:].bitcast(F32R), wx[128 * k:128 * (k + 1), :].bitcast(F32R))
                ps2 = p2.enter_context(tc.tile_pool(name="ps2", bufs=2, space="PSUM"))
                xdp = p2.enter_context(tc.tile_pool(name="xdp", bufs=2))
                for c in range(NPC):
                    lsl = slice(PC * c, PC * (c + 1))
                    pxd = ps2.tile([96, PC], F32, tag="pxd")
                    for k in range(KT):
                        nc.tensor.matmul(pxd[:], wx_t[k][:].bitcast(F32R),
                                         xs_t[k][:, lsl].bitcast(F32R),
                                         start=(k == 0), stop=(k == KT - 1))
                    xt = xdp.tile([96, PC], F32, tag="xdp")
                    nc.scalar.copy(xt[:], pxd[:])
                    nc.sync.dma_start(xd_bounce[c][:], xt[:])
                    if use_collective:
                        nc.gpsimd.collective_compute(
                            "AllReduce", OP.add,
                            replica_groups=[[0, 1, 2, 3], [4, 5, 6, 7]],
                            ins=[xd_bounce[c].opt()], outs=[xd_red[c].opt()])
                    else:
                        nc.sync.dma_start(xd_red[c][:], xd_bounce[c][:])
            # B/C rows staged at partitions 0..31 for the selection broadcasts
            bcT_p = ctx.enter_context(tc.tile_pool(name="bcT", bufs=1))
            bcT = bcT_p.tile([2 * DS, L], F32, tag="bcT")
            for c in range(NPC):
                nc.sync.dma_start(bcT[:, PC * c:PC * (c + 1)].bitcast(F32R),
                                  xd_red[c][DR:DR + 2 * DS, :].bitcast(F32R))

            # ---------------- Phase 3: delta = softplus(dt) ----------------
            with ExitStack() as p3:
                wdt_p = p3.enter_context(tc.tile_pool(name="wdt", bufs=1))
                wdt_t = wdt_p.tile([128, DL], F32, tag="wdt")
                nc.sync.dma_start(wdt_t[0:DR, :].bitcast(F32R), wdt[:].bitcast(F32R))
                ps3 = p3.enter_context(tc.tile_pool(name="ps3", bufs=2, space="PSUM"))
                dchunk = p3.enter_context(tc.tile_pool(name="dch", bufs=2))
                xdb_p = p3.enter_context(tc.tile_pool(name="xdb", bufs=2))
                for c in range(NPC):
                    lsl = slice(PC * c, PC * (c + 1))
                    xdb = xdb_p.tile([DR, PC], F32, tag="xdb")
                    nc.sync.dma_start(xdb[:].bitcast(F32R), xd_red[c][0:DR, :].bitcast(F32R))
                    for k in range(KT):
                        pdt = ps3.tile([128, PC], F32, tag="pdt")
                        nc.tensor.matmul(pdt[:],
                                         wdt_t[0:DR, 128 * k:128 * (k + 1)].bitcast(F32R),
                                         xdb[:].bitcast(F32R), start=True, stop=True)
                        dt = dchunk.tile([128, PC], F32, tag="dt")
                        nc.scalar.activation(dt[:], pdt[:], AF.Exp,
                                             bias=bdt_t[:, k:k + 1])
                        nc.scalar.activation(dt[:], dt[:], AF.Ln, bias=1.0)
                        nc.sync.dma_start(delta_dram[128 * k:128 * (k + 1), lsl], dt[:])

            # ---------------- Phase M: scan middle + output ----------------
            mid = ctx.enter_context(tc.tile_pool(name="mid", bufs=CFG.get("mid_bufs", 2)))
            s_pool = ctx.enter_context(tc.tile_pool(name="spool", bufs=CFG.get("s_bufs", 1)))
            bc_p = ctx.enter_context(tc.tile_pool(name="bc", bufs=1))
            ps_bc = ctx.enter_context(tc.tile_pool(name="psbc", bufs=2, space="PSUM"))
            ps_out = ctx.enter_context(tc.tile_pool(name="psout", bufs=CFG.get("psout_bufs", 2), space="PSUM"))
            dre = ctx.enter_context(tc.tile_pool(name="dre", bufs=CFG.get("dre_bufs", 2)))
            ych = ctx.enter_context(tc.tile_pool(name="ych", bufs=CFG.get("ych_bufs", 2)))
            outc = ctx.enter_context(tc.tile_pool(name="outc", bufs=1))

            for c in range(NCH):
                lsl = slice(T * c, T * (c + 1))
                # B_bc / C_bc patterns [128, 16*T], (n-outer, l-inner)
                b_bc = bc_p.tile([128, DS * T], F32, tag="b_bc")
                c_bc = bc_p.tile([128, DS * T], F32, tag="c_bc")
                for half, dst in (() if 'bc' in skip else ((0, b_bc), (1, c_bc))):
                    if CFG.get("bc_pb", True):
                        for n in range(DS):
                            nc.gpsimd.partition_broadcast(
                                dst[:, T * n:T * (n + 1)],
                                bcT[16 * half + n:16 * half + n + 1, lsl])
                        continue
                    for g in range(DS // 2):  # two n per psum tile
                        pb = ps_bc.tile([128, 2 * T], F32, tag="pb")
                        for j in range(2):
                            n = 16 * half + 2 * g + j
                            nc.tensor.matmul(pb[:, T * j:T * (j + 1)],
                                             sel_t[:, 128 * n:128 * (n + 1)].bitcast(F32R),
                                             bcT[:, lsl].bitcast(F32R),
                                             start=True, stop=True)
                        (nc.scalar.copy if CFG["bccopy"] == "a" else
                         nc.gpsimd.tensor_copy)(dst[:, 2 * T * g:2 * T * (g + 1)], pb[:])

                for k in range(KT):
                    ksl = slice(128 * k, 128 * (k + 1))
                    nsl = slice(DS * k, DS * (k + 1))
                    d_ch = dre.tile([128, T], F32, tag="dch")
                    nc.sync.dma_start(d_ch[:], delta_dram[ksl, lsl])
                    # dA[n*T+l] = exp(A[d,n] * delta[d,l])
                    dA = mid.tile([128, DS * T], F32, tag="dA")
                    if 'dA' not in skip:
                      for n in range(DS):
                        nc.scalar.activation(dA[:, T * n:T * (n + 1)], d_ch[:],
                                             AF.Exp,
                                             scale=asc_t[:, DS * k + n:DS * k + n + 1])
                    # dBu = (delta*x) expanded  *  B_bc
                    dx = dre.tile([128, T], F32, tag="dx")
                    nc.vector.tensor_tensor(dx[:], d_ch[:], xs_t[k][:, lsl], OP.mult)
                    dBu = mid.tile([128, DS * T], F32, tag="dBu")
                    if 'dbu' not in skip:
                        engd = nc.vector if CFG["dbu"] == "v" else nc.gpsimd
                        engd.tensor_tensor(
                            dBu[:].rearrange("p (n l) -> p n l", n=DS),
                            dx[:, None, :].to_broadcast([128, DS, T]),
                            b_bc[:].rearrange("p (n l) -> p n l", n=DS),
                            OP.mult)
                    # fold carried state into first column of each segment
                    fx = dre.tile([128, DS], F32, tag="fx")
                    nc.vector.tensor_tensor(fx[:], dA[:, 0:DS * T:T],
                                            state_t[:, nsl], OP.mult)
                    nc.vector.tensor_tensor(dBu[:, 0:DS * T:T], dBu[:, 0:DS * T:T],
                                            fx[:], OP.add)
                    nc.vector.memset(dA[:, 0:DS * T:T], 0.0)
                    # the scan: state = dA*state + dBu over the whole (n,l) axis
                    s_t = s_pool.tile([128, DS * T], F32, tag="s")
                    if 'scan' not in skip:
                        se = (nc.gpsimd if (CFG.get("scan_split") and
                                            k >= CFG["scan_split"]) else nc.vector)
                        se.tensor_tensor_scan(s_t[:], dA[:], dBu[:], 0.0,
                                              OP.mult, OP.add)
                        se.tensor_copy(state_t[:, nsl], s_t[:, T - 1:DS * T:T])
                    # y = sum_n s*C
                    if 'cmul' not in skip:
                        eng = nc.vector if CFG["cmul"] == "v" else nc.gpsimd
                        eng.tensor_tensor(s_t[:], s_t[:], c_bc[:], OP.mult)
                    y_r = ych.tile([128, T], F32, tag="yr")
                    if 'reduce' not in skip:
                        nc.vector.tensor_reduce(
                            y_r[:], s_t[:].rearrange("p (n l) -> p l n", n=DS),
                            axis=mybir.AxisListType.X, op=OP.add)
                    # skip + gate
                    nc.vector.scalar_tensor_tensor(
                        y_r[:], xs_t[k][:, lsl], dpar_t[:, k:k + 1], y_r[:],
                        OP.mult, OP.add)
                    z_ch = dre.tile([128, T], F32, tag="zch")
                    nc.sync.dma_start(z_ch[:], z_dram[ksl, lsl])
                    g_t = ych.tile([128, T], F32, tag=f"g{k}")
                    nc.vector.tensor_tensor(g_t[:].bitcast(F32R), y_r[:], z_ch[:], OP.mult)
                    if k == 0:
                        g_list = []
                    g_list.append(g_t)

                # out_proj for this chunk: out[l, :] += g^T @ W_out
                for h in range(0 if 'out' in skip else T // 128):
                    po0 = ps_out.tile([128, 512], F32, tag="po0")
                    po1 = ps_out.tile([128, 512], F32, tag="po1")
                    msl = slice(128 * h, 128 * (h + 1))
                    for k in range(KT):
                        nc.tensor.matmul(po0[:], g_list[k][:, msl].bitcast(F32R),
                                         wout_t[k][:, 0:512].bitcast(F32R),
                                         start=(k == 0), stop=(k == KT - 1))
                    for k in range(KT):
                        nc.tensor.matmul(po1[:], g_list[k][:, msl].bitcast(F32R),
                                         wout_t[k][:, 512:1024].bitcast(F32R),
                                         start=(k == 0), stop=(k == KT - 1))
                    ot = outc.tile([128, DM], F32, tag="ot")
                    nc.scalar.copy(ot[:, 0:512], po0[:])
                    nc.scalar.copy(ot[:, 512:1024], po1[:])
                    nc.sync.dma_start(outp[T * c + 128 * h:T * c + 128 * (h + 1), :],
                                      ot[:])
        for _rep in range(n_reps):
            emit_once()
    nc.compile()
    return nc


_NC_CACHE = None


def kernel(**inputs) -> np.ndarray:
    global _NC_CACHE
    hs = np.ascontiguousarray(inputs["hidden_states"], np.float32)
    W_in = np.asarray(inputs["W_in"], np.float32)
    W_conv = np.asarray(inputs["W_conv"], np.float32)
    b_conv = np.asarray(inputs["b_conv"], np.float32)
    W_x = np.asarray(inputs["W_x"], np.float32)
    W_dt = np.asarray(inputs["W_dt"], np.float32)
    b_dt = np.asarray(inputs["b_dt"], np.float32)
    A_log = np.asarray(inputs["A_log"], np.float32)
    D_param = np.asarray(inputs["D_param"], np.float32)
    W_out = np.asarray(inputs["W_out"], np.float32)
    A = -np.exp(A_log.astype(np.float64)).astype(np.float32)    # (DI, DS)
    sel_mat = np.zeros((2 * DS, 2 * DS * 128), np.float32)
    for n in range(2 * DS):
        sel_mat[n, 128 * n:128 * (n + 1)] = 1.0

    in_maps = []
    for cid in range(8):
        b, s = cid // NSH, cid % NSH
        sh = slice(DL * s, DL * (s + 1))
        in_maps.append({
            "hsT": np.ascontiguousarray(hs[b].T),
            "wix": np.ascontiguousarray(W_in[:, 2 * DL * s:2 * DL * (s + 1):2]),
            "wiz": np.ascontiguousarray(W_in[:, 2 * DL * s + 1:2 * DL * (s + 1) + 1:2]),
            "wc": np.ascontiguousarray(W_conv[:, 0, sh].T),
            "bcv": np.ascontiguousarray(b_conv[sh].reshape(DL, 1)),
            "nbcv": np.ascontiguousarray(-b_conv[sh].reshape(DL, 1)),
            "wx": np.ascontiguousarray(W_x[sh, :]),
            "wdt": np.ascontiguousarray(W_dt[:, sh]),
            "bdt": np.ascontiguousarray(b_dt[sh].reshape(DL, 1)),
            "asc": np.ascontiguousarray(A[sh, :]),
            "dpar": np.ascontiguousarray(D_param[sh].reshape(DL, 1)),
            "wout": np.ascontiguousarray(W_out[sh, :]),
            "sel": sel_mat,
        })

    global _LAST_IN_MAPS
    _LAST_IN_MAPS = in_maps
    if _NC_CACHE is None:
        _NC_CACHE = build_program()
    res = run_bass_kernel_spmd(_NC_CACHE, in_maps, list(range(8)))
    out = np.zeros((B, L, DM), np.float32)
    for cid in range(8):
        out[cid // NSH] += res.results[cid]["outp"]
    return out


if __name__ == "__main__":
    rng = np.random.default_rng(0)
    dummy = {
        "hidden_states": rng.standard_normal((B, L, DM), dtype=np.float32),
        "W_in": rng.standard_normal((DM, 2 * DI), dtype=np.float32) * 0.03,
        "W_conv": rng.standard_normal((DC, 1, DI), dtype=np.float32) * 0.5,
        "b_conv": np.zeros((DI,), np.float32),
        "W_x": rng.standard_normal((DI, DR + 2 * DS), dtype=np.float32) * 0.02,
        "W_dt": rng.standard_normal((DR, DI), dtype=np.float32) * 0.12,
        "b_dt": rng.standard_normal((DI,), dtype=np.float32) * 0.01,
        "A_log": np.log(np.broadcast_to(np.arange(1, DS + 1, dtype=np.float32),
                                        (DI, DS))).copy(),
        "D_param": np.ones((DI,), np.float32),
        "W_out": rng.standard_normal((DI, DM), dtype=np.float32) * 0.03,
    }
    out = kernel(**dummy)
    print("out", out.shape, out.dtype, np.abs(out).max())

